# revision 27
# baseline (speedup 1.0000x reference)
"""DigitCaps dynamic-routing kernel for 8 Trainium2 NeuronCores.

Math (reference):
  u_hat[b,i,j,d] = sum_e W[0,i,j,d,e] * x[b,i,e]
  2 routing iterations; iteration 1 has b=0 so c = 1/32 exactly:
    s1 = (1/32) sum_i u_hat ;  v1 = squash(s1)
    b2 = sum_d u_hat * v1    ;  c2 = softmax_j(b2)
    s2 = sum_i c2 * u_hat    ;  v2 = squash(s2)   -> output

Sharding: in_capsules (i) split 8 ways (256 per core); batch b=128 lives in
the partition dimension everywhere.  The two reductions over i (s1, s2) are
[128,512] fp32 AllReduces.  Softmax/b2 are i-local, so no other comms.

Per-core layouts (host-prepped, zero math on host — pure transpose):
  wt[r, e, g, jd] : [4, 8, 64, 512]  W for i = core*256 + g*4 + r
  xt[r, e, g, b]  : [4, 8, 64, 128]  x likewise
On SBUF each strip r sits at partition base 32*r (rows e=0..7 used), so the
K=8 matmuls auto-derive tile_position=(32r,0) and run 4-way row-tiled.
"""

import sys
for _p in ("/opt/pypackages", "/opt/trn_rl_repo"):
    if _p not in sys.path:
        sys.path.insert(0, _p)

import numpy as np

import concourse.bass as bass
import concourse.bacc as bacc
import concourse.tile as tile
from concourse import mybir
from concourse.bass_utils import run_bass_kernel_spmd

B = 128
I = 2048
E = 8
J = 32
D = 16
JD = J * D          # 512
NC_ = 8             # cores
IS = I // NC_       # 256 in_caps per core
NG = IS // 4        # 64 groups of 4 strips
WTILE_G = 8         # groups per streamed W tile
EPS = 1e-8

f32 = mybir.dt.float32
f32r = mybir.dt.float32r
bf16 = mybir.dt.bfloat16


def _bc(ap, n):
    """Broadcast an AP along a new innermost dim of size n (step 0)."""
    return bass.AP(tensor=ap.tensor, offset=ap.offset, ap=[*ap.ap, [0, n]])


def _squash(nc, pool, s_sb, v_sb):
    """v = |s|^2/(1+|s|^2) * s/(|s|+eps), norm over d within each j."""
    sq = pool.tile([B, JD], f32, tag="sqs")
    nc.vector.tensor_mul(sq[:, :], s_sb[:, :], s_sb[:, :])
    n2 = pool.tile([B, J], f32, tag="sqn2")
    nc.vector.tensor_reduce(
        out=n2[:, :], in_=sq[:, :].rearrange("p (j d) -> p j d", d=D),
        axis=mybir.AxisListType.X, op=mybir.AluOpType.add)
    nrm = pool.tile([B, J], f32, tag="sqn")
    nc.scalar.sqrt(out=nrm[:, :], in_=n2[:, :])
    a1 = pool.tile([B, J], f32, tag="sqa")
    nc.vector.tensor_scalar_add(a1[:, :], n2[:, :], 1.0)
    b1 = pool.tile([B, J], f32, tag="sqb")
    nc.vector.tensor_scalar_add(b1[:, :], nrm[:, :], EPS)
    den = pool.tile([B, J], f32, tag="sqden")
    nc.vector.tensor_mul(den[:, :], a1[:, :], b1[:, :])
    rden = pool.tile([B, J], f32, tag="sqrden")
    nc.vector.reciprocal(out=rden[:, :], in_=den[:, :])
    sc = pool.tile([B, J], f32, tag="sqsc")
    nc.vector.tensor_mul(sc[:, :], n2[:, :], rden[:, :])
    nc.vector.tensor_tensor(
        out=v_sb[:, :].rearrange("p (j d) -> p j d", d=D),
        in0=s_sb[:, :].rearrange("p (j d) -> p j d", d=D),
        in1=_bc(sc[:, :], D), op=mybir.AluOpType.mult)


def build_nc(num_devices=NC_, with_cc=True):
    nc = bacc.Bacc("TRN2", target_bir_lowering=False, debug=False,
                   num_devices=num_devices)
    wt = nc.dram_tensor("wt", [4, E, NG, JD], f32r, kind="ExternalInput")
    xt = nc.dram_tensor("xt", [4, E, NG, B], f32r, kind="ExternalInput")
    yout = nc.dram_tensor("yout", [B, JD], f32, kind="ExternalOutput")

    cc1_in = nc.dram_tensor("cc1_in", [B, JD], f32)
    cc1_out = nc.dram_tensor("cc1_out", [B, JD], f32, addr_space="Shared")
    cc2_in = nc.dram_tensor("cc2_in", [B, JD], f32)
    cc2_out = nc.dram_tensor("cc2_out", [B, JD], f32, addr_space="Shared")
    rgroups = [list(range(num_devices))]

    def allreduce(cin, cout):
        if with_cc:
            nc.gpsimd.collective_compute(
                "AllReduce", mybir.AluOpType.add, replica_groups=rgroups,
                ins=[cin.ap()], outs=[cout.ap()])
        else:
            nc.sync.dma_start(out=cout.ap(), in_=cin.ap())

    with tile.TileContext(nc) as tc:
        with (
            tc.tile_pool(name="consts", bufs=1) as consts,
            tc.tile_pool(name="wpool", bufs=2) as wpool,
            tc.tile_pool(name="mpool", bufs=2) as mpool,
            tc.tile_pool(name="spool", bufs=2) as spool,
        ):
            # ---- load x (stationary) : strips at partition base 32r ----
            xg = consts.tile([128, NG, B], f32r, tag="xg")
            for r in range(4):
                nc.sync.dma_start(out=xg[32 * r:32 * r + E, :, :],
                                  in_=xt[r, :, :, :])

            # ---- pass 1: s1_part = sum_i u_hat_i  (PSUM accumulation) ----
            p1_cm = tc.tile_pool(name="p1", bufs=1, space="PSUM")
            p1 = p1_cm.__enter__()
            s1p = [p1.tile([B, JD], f32, tag=f"s1p{r}", name=f"s1p{r}")
                   for r in range(4)]
            for wti in range(NG // WTILE_G):
                wtile = wpool.tile([128, WTILE_G, JD], f32r, tag="wt")
                for r in range(4):
                    nc.sync.dma_start(
                        out=wtile[32 * r:32 * r + E, :, :],
                        in_=wt[r, :, wti * WTILE_G:(wti + 1) * WTILE_G, :])
                for gg in range(WTILE_G):
                    g = wti * WTILE_G + gg
                    for r in range(4):
                        nc.tensor.matmul(
                            out=s1p[r][:, :],
                            lhsT=xg[32 * r:32 * r + E, g, :],
                            rhs=wtile[32 * r:32 * r + E, gg, :],
                            start=(g == 0), stop=(g == NG - 1),
                            tile_position=(32 * r, 0))
            # combine 4 strip-partials (one PSUM operand per DVE op)
            t01 = spool.tile([B, JD], f32, tag="t01")
            ssum = consts.tile([B, JD], f32, tag="ssum")
            nc.vector.tensor_copy(t01[:, :], s1p[0][:, :])
            nc.vector.tensor_add(t01[:, :], t01[:, :], s1p[1][:, :])
            nc.vector.tensor_add(t01[:, :], t01[:, :], s1p[2][:, :])
            nc.vector.tensor_add(ssum[:, :], t01[:, :], s1p[3][:, :])
            p1_cm.__exit__(None, None, None)

            # ---- AllReduce s1 over 8 cores ----
            nc.sync.dma_start(out=cc1_in.ap(), in_=ssum[:, :])
            allreduce(cc1_in, cc1_out)
            s1 = consts.tile([B, JD], f32, tag="s1")
            nc.sync.dma_start(out=s1[:, :], in_=cc1_out.ap())
            nc.vector.tensor_scalar_mul(s1[:, :], s1[:, :], 1.0 / J)

            # ---- v1 = squash(s1) ----
            v1 = consts.tile([B, JD], f32, tag="v1")
            _squash(nc, spool, s1, v1)
            # bf16 v1 replicated over the 4 strips (enables 2x DVE mode)
            v1x4 = consts.tile([B, 4, JD], bf16, tag="v1x4")
            for r in range(4):
                nc.vector.tensor_copy(v1x4[:, r, :], v1[:, :])

            # ---- pass 2: routing iteration 2, i-local ----
            # Per group of 4 capsules: one [B, 4*JD] fused op per stage.
            up_cm = tc.tile_pool(name="up", bufs=2, space="PSUM")
            up = up_cm.__enter__()
            s2a = [consts.tile([B, 4, JD], f32, tag=f"s2a{h}",
                               name=f"s2a{h}") for h in range(2)]
            nc.gpsimd.memset(s2a[0][:, :, :], 0.0)
            nc.vector.memset(s2a[1][:, :, :], 0.0)
            for wti in range(NG // WTILE_G):
                wtile = wpool.tile([128, WTILE_G, JD], f32r, tag="wt")
                for r in range(4):
                    nc.sync.dma_start(
                        out=wtile[32 * r:32 * r + E, :, :],
                        in_=wt[r, :, wti * WTILE_G:(wti + 1) * WTILE_G, :])
                for gg in range(WTILE_G):
                    g = wti * WTILE_G + gg
                    # u_hat for the 4 strips: one PSUM tile, 4 row-tiled MMs
                    u4 = up.tile([B, 4, JD], f32, tag="u4")
                    for r in range(4):
                        nc.tensor.matmul(
                            out=u4[:, r, :],
                            lhsT=xg[32 * r:32 * r + E, g, :],
                            rhs=wtile[32 * r:32 * r + E, gg, :],
                            start=True, stop=True,
                            tile_position=(32 * r, 0))
                    # bf16 copy of u_hat (ACT) -> 2x DVE mode downstream
                    usb = mpool.tile([B, 4, JD], bf16, tag="usb", bufs=4)
                    nc.scalar.copy(out=usb[:, :, :], in_=u4[:, :, :])
                    # m = u*v1 ; b2[b, (r j)] = sum_d m
                    m = mpool.tile([B, 4, JD], bf16, tag="m")
                    nc.vector.tensor_mul(m[:, :, :], usb[:, :, :],
                                         v1x4[:, :, :])
                    b2g = spool.tile([B, 4 * J], f32, tag="b2g")
                    nc.vector.tensor_reduce(
                        out=b2g[:, :],
                        in_=m[:, :, :].rearrange("p r (j d) -> p (r j) d",
                                                 d=D),
                        axis=mybir.AxisListType.X, op=mybir.AluOpType.add)
                    # softmax over j for the 4 capsules at once
                    eg = spool.tile([B, 4 * J], f32, tag="eg")
                    nc.scalar.activation(
                        out=eg[:, :], in_=b2g[:, :],
                        func=mybir.ActivationFunctionType.Exp)
                    z = spool.tile([B, 4], f32, tag="z")
                    nc.vector.tensor_reduce(
                        out=z[:, :],
                        in_=eg[:, :].rearrange("p (r j) -> p r j", j=J),
                        axis=mybir.AxisListType.X, op=mybir.AluOpType.add)
                    rz = spool.tile([B, 4], f32, tag="rz")
                    nc.vector.reciprocal(out=rz[:, :], in_=z[:, :])
                    c2g = spool.tile([B, 4, J], f32, tag="c2g")
                    nc.vector.tensor_tensor(
                        out=c2g[:, :, :],
                        in0=eg[:, :].rearrange("p (r j) -> p r j", j=J),
                        in1=_bc(rz[:, :], J), op=mybir.AluOpType.mult)
                    # s2 += c2 * u_hat : split groups DVE / GPSIMD (~40/60,
                    # GPSIMD is ~2x slower per op but otherwise idle)
                    h = 0 if (g % 8) < 3 else 1
                    eng = nc.vector if h == 0 else nc.gpsimd
                    t = mpool.tile([B, 4, J, D], f32, tag=f"t{h}",
                                   name=f"t{h}")
                    eng.tensor_tensor(
                        out=t[:, :, :, :],
                        in0=usb[:, :, :].rearrange("p r (j d) -> p r j d",
                                                   d=D),
                        in1=_bc(c2g[:, :, :], D), op=mybir.AluOpType.mult)
                    eng.tensor_add(
                        s2a[h][:, :, :],
                        s2a[h][:, :, :],
                        t[:, :, :, :].rearrange("p r j d -> p r (j d)"))
            # fold the two [B, 4, JD] accumulators down to [B, JD]
            s2r = [spool.tile([B, JD], f32, tag=f"s2r{h}", name=f"s2r{h}")
                   for h in range(2)]
            for h in range(2):
                nc.vector.tensor_reduce(
                    out=s2r[h][:, :],
                    in_=s2a[h][:, :, :].rearrange("p r c -> p c r"),
                    axis=mybir.AxisListType.X, op=mybir.AluOpType.add)
            s2part = consts.tile([B, JD], f32, tag="s2part")
            nc.vector.tensor_add(s2part[:, :], s2r[0][:, :], s2r[1][:, :])
            up_cm.__exit__(None, None, None)

            # ---- AllReduce s2 ----
            nc.sync.dma_start(out=cc2_in.ap(), in_=s2part[:, :])
            allreduce(cc2_in, cc2_out)
            s2 = consts.tile([B, JD], f32, tag="s2")
            nc.sync.dma_start(out=s2[:, :], in_=cc2_out.ap())

            # ---- v2 = squash(s2) -> output ----
            v2 = consts.tile([B, JD], f32, tag="v2")
            _squash(nc, spool, s2, v2)
            nc.sync.dma_start(out=yout.ap(), in_=v2[:, :])

    nc.compile()
    return nc


_NC_CACHE = None


def _get_nc():
    global _NC_CACHE
    if _NC_CACHE is None:
        _NC_CACHE = build_nc()
    return _NC_CACHE


def _prep_inputs(x, W):
    """Pure layout transform: returns per-core wt, xt arrays."""
    # W[0]: [I, J, D, E] -> [core, g, r, j, d, e] -> [core, r, e, g, (j d)]
    Wv = np.ascontiguousarray(W[0]).reshape(NC_, NG, 4, J, D, E)
    wt = np.ascontiguousarray(Wv.transpose(0, 2, 5, 1, 3, 4)).reshape(
        NC_, 4, E, NG, JD)
    # x: [B, I, E] -> [i, e, b] -> [core, g, r, e, b] -> [core, r, e, g, b]
    xv = np.ascontiguousarray(x.transpose(1, 2, 0)).reshape(
        NC_, NG, 4, E, B)
    xt = np.ascontiguousarray(xv.transpose(0, 2, 3, 1, 4))
    return wt, xt


def run(x, W, trace=False, **kw):
    x = np.asarray(x, dtype=np.float32)
    W = np.asarray(W, dtype=np.float32)
    wt, xt = _prep_inputs(x, W)
    nc = _get_nc()
    in_maps = [{"wt": wt[c], "xt": xt[c]} for c in range(NC_)]
    res = run_bass_kernel_spmd(nc, in_maps, core_ids=list(range(NC_)),
                               trace=trace, **kw)
    out = res.results[0]["yout"].reshape(B, J, D).astype(np.float32)
    return out, res


def kernel(x, W):
    out, _ = run(x, W)
    return out
